# revision 21
# baseline (speedup 1.0000x reference)
"""Trainium2 Bass kernel for a dense transformer block (B=2, T=2048, C=1024,
H=16 heads, HS=64, FF=4096, fp32), SPMD across 8 NeuronCores.

Sharding strategy (v2)
----------------------
- LayerNorms + FFN + proj: sequence-parallel; core c owns 512 tokens.
- Attention: head-parallel; core c owns heads 2c, 2c+1 over all tokens.
- QKV is computed token-sharded (each core projects its OWN 512 tokens
  through ALL 16 heads' Q/K/V), so the PE has work during the runtime's
  collective-stream init barrier; results are re-sharded head-wise with
  three pipelined 1MB AllToAlls (V^T first - PV needs it earliest - then
  qk-even-heads, qk-odd-heads) instead of one serial 8MB AllGather.
- att^T returns to token-sharding with one AllToAll per local head; the
  even-head half of the output projection (host-permuted Wproj rows)
  overlaps the second AllToAll.

Numerics: matmul operands bf16 (fp32 PSUM accumulate); LayerNorm stats,
softmax normalization, residuals in fp32. LN scale/bias and the per-head
attention scale fold into the weights on the host; K-bias dropped
(softmax is invariant to per-query constant offsets).

All weights and x arrive host-packed in [partition, tile, cols] layout so
every DMA is 128 large contiguous descriptors (no rearrange storms).
Causal masking is a DVE multiply with four precomputed [128,512] 0/1
tiles; the softmax denominator reciprocal is broadcast across partitions
with a GpSimd partition_broadcast.
"""

import os
import numpy as np

B, T, C = 2, 2048, 1024
H, HS = 16, 64
FF = 4 * C
EPS = 1e-5
NCORE = 8
TOK = B * T            # 4096 flattened tokens
CHUNK = TOK // NCORE   # 512 tokens per core
P = 128
NTT = CHUNK // P       # 4 token tiles of 128 per core
NG = C // P            # 8 channel chunks
NF = FF // P           # 32 ff slices
LH = 2                 # local heads per core

_BUILT = None


def _build():
    import concourse.bass as bass
    import concourse.tile as tile
    from bass_rust import add_dep_helper
    from concourse import bacc, mybir
    from concourse.masks import make_identity
    from contextlib import ExitStack

    f32 = mybir.dt.float32
    bf16 = mybir.dt.bfloat16
    Alu = mybir.AluOpType
    Act = mybir.ActivationFunctionType

    nc = bacc.Bacc("TRN2", target_bir_lowering=False, debug=False,
                   num_devices=NCORE)

    xc = nc.dram_tensor("xc", [P, NTT, C], f32, kind="ExternalInput").ap()
    # wqkv blocks: [0]=v (heads 2d,2d+1 per dst d), [1]=qk head 2d, [2]=qk 2d+1
    wqkv = nc.dram_tensor("wqkv", [3, P, NG, C], bf16,
                          kind="ExternalInput").ap()
    bqk0 = nc.dram_tensor("bqk0", [P, NCORE], f32, kind="ExternalInput").ap()
    bqk1 = nc.dram_tensor("bqk1", [P, NCORE], f32, kind="ExternalInput").ap()
    bv = nc.dram_tensor("bv", [P, NCORE], f32, kind="ExternalInput").ap()
    wproj = nc.dram_tensor("wproj", [P, NG, C], bf16,
                           kind="ExternalInput").ap()
    w1 = nc.dram_tensor("w1", [P, NG, FF], bf16, kind="ExternalInput").ap()
    bff1 = nc.dram_tensor("bff1", [P, NF], f32, kind="ExternalInput").ap()
    w2 = nc.dram_tensor("w2", [2, 4, P, 8, 512], bf16,
                        kind="ExternalInput").ap()
    out = nc.dram_tensor("out", [CHUNK, C], f32, kind="ExternalOutput").ap()
    DEBUG = bool(int(os.environ.get("BASSK_DEBUG", "0")))
    if DEBUG:
        dbg_qkT = nc.dram_tensor("dbg_qkT", [2, 2, 64, TOK], bf16,
                                 kind="ExternalOutput").ap()
        dbg_v = nc.dram_tensor("dbg_v", [P, TOK // P, 132], bf16,
                               kind="ExternalOutput").ap()
        dbg_xmid = nc.dram_tensor("dbg_xmid", [P, NTT, C], f32,
                                  kind="ExternalOutput").ap()

    qk0_b = nc.dram_tensor("qk0_b", [NCORE, P, CHUNK], bf16)
    qk1_b = nc.dram_tensor("qk1_b", [NCORE, P, CHUNK], bf16)
    v_b = nc.dram_tensor("v_b", [NCORE, P, NTT, 132], bf16)
    qk0_r = nc.dram_tensor("qk0_r", [NCORE, P, CHUNK], bf16)
    qk1_r = nc.dram_tensor("qk1_r", [NCORE, P, CHUNK], bf16)
    v_r = nc.dram_tensor("v_r", [NCORE, P, NTT, 132], bf16)
    attT_bounce = [nc.dram_tensor(f"attT_bounce{i}", [NCORE, 64, CHUNK], bf16)
                   for i in range(LH)]
    attT_recv = [nc.dram_tensor(f"attT_recv{i}", [NCORE, 64, CHUNK], bf16)
                 for i in range(LH)]
    groups = [list(range(NCORE))]

    with tile.TileContext(nc) as tc, ExitStack() as top:
        const = top.enter_context(tc.tile_pool(name="const", bufs=1))
        persist = top.enter_context(tc.tile_pool(name="persist", bufs=1))
        ps = top.enter_context(tc.tile_pool(name="ps", bufs=4, space="PSUM"))
        ps2 = top.enter_context(tc.tile_pool(name="ps2", bufs=4, space="PSUM"))

        ident = const.tile([P, P], bf16)
        make_identity(nc, ident)
        eps_sb = const.tile([P, 1], f32)
        nc.vector.memset(eps_sb, EPS)
        # causal mask tiles: mask[m][p, col] = col >= p + 128*m
        masks = const.tile([P, 4, 512], bf16)
        nc.vector.memset(masks, 1.0)
        for m in range(4):
            nc.gpsimd.affine_select(
                out=masks[:, m, :], in_=masks[:, m, :], pattern=[[1, 512]],
                compare_op=Alu.is_ge, fill=0.0, base=-128 * m,
                channel_multiplier=-1)

        xc_sb = persist.tile([P, NTT, C], f32)
        xmid_sb = persist.tile([P, NTT, C], f32)
        bqk0_sb = persist.tile([P, NCORE], f32)
        bqk1_sb = persist.tile([P, NCORE], f32)
        bv_sb = persist.tile([P, NCORE], f32)
        bff1_sb = persist.tile([P, NF], f32)
        w1p = top.enter_context(tc.tile_pool(name="w1p", bufs=1))
        w1_sb = w1p.tile([P, NG, FF], bf16)   # prefetched during attention
        prp = top.enter_context(tc.tile_pool(name="prp", bufs=1))
        wpp = top.enter_context(tc.tile_pool(name="wpp", bufs=1))
        ats0 = prp.tile([P, 4, CHUNK], bf16, name="ats0")
        ats1 = prp.tile([P, 4, CHUNK], bf16, name="ats1")
        wp = wpp.tile([P, NG, C], bf16)

        for jt in range(NTT):
            nc.sync.dma_start(out=xc_sb[:, jt, :], in_=xc[:, jt, :])
        nc.sync.dma_start(out=bqk0_sb, in_=bqk0)
        nc.sync.dma_start(out=bqk1_sb, in_=bqk1)
        nc.sync.dma_start(out=bv_sb, in_=bv)
        nc.sync.dma_start(out=bff1_sb, in_=bff1)

        def layernorm_tile(pool, src_ap, out_dt):
            """src_ap: [P, C] fp32 in SBUF -> normalized [P, C] tile."""
            stats = pool.tile([P, 2, 6], f32, tag="ln_stats")
            nc.vector.bn_stats(out=stats[:, 0, :], in_=src_ap[:, 0:512])
            nc.vector.bn_stats(out=stats[:, 1, :], in_=src_ap[:, 512:1024])
            mv = pool.tile([P, 2], f32, tag="ln_mv")
            nc.vector.bn_aggr(out=mv, in_=stats)
            rstd = pool.tile([P, 1], f32, tag="ln_rstd")
            nc.scalar.activation(rstd, mv[:, 1:2], Act.Sqrt, bias=eps_sb)
            nc.vector.reciprocal(rstd, rstd)
            negmr = pool.tile([P, 1], f32, tag="ln_negmr")
            nc.vector.tensor_scalar(negmr, mv[:, 0:1], rstd, -1.0,
                                    Alu.mult, Alu.mult)
            hn = pool.tile([P, C], out_dt, tag="ln_out")
            nc.scalar.activation(hn, src_ap, Act.Identity,
                                 bias=negmr, scale=rstd)
            return hn

        # ---------------- Stage A: LN1 + transpose (own chunk) --------------
        with ExitStack() as sa:
            lnp = sa.enter_context(tc.tile_pool(name="lnp", bufs=3))
            qkvp = sa.enter_context(tc.tile_pool(name="qkvp", bufs=1))
            wqp = sa.enter_context(tc.tile_pool(name="wqp", bufs=2))

            hT_sb = qkvp.tile([P, NG, CHUNK], bf16)
            wv_sb = wqp.tile([P, NG, C], bf16, tag="wblk", name="wv_sb")
            nc.sync.dma_start(out=wv_sb, in_=wqkv[0])
            wq0_sb = wqp.tile([P, NG, C], bf16, tag="wblk", name="wq0_sb")
            nc.sync.dma_start(out=wq0_sb, in_=wqkv[1])

            for jt in range(NTT):
                hn = layernorm_tile(lnp, xc_sb[:, jt, :], bf16)
                for g in range(NG):
                    tp = ps.tile([P, P], bf16, tag="bank")
                    nc.tensor.transpose(tp, hn[:, P * g:P * (g + 1)], ident)
                    nc.vector.tensor_copy(
                        hT_sb[:, g, P * jt:P * (jt + 1)], tp)

            # -------- Stage A2: QKV for all heads, 3 pipelined AllToAlls ----
            stg = sa.enter_context(tc.tile_pool(name="stg", bufs=2))

            # phase 0: V (both heads of dst d), pre-transposed to [token, ch]
            vT_st = stg.tile([P, NCORE, NTT, 132], bf16, tag="stg",
                             name="vT_st")
            nc.vector.memset(vT_st.rearrange(
                "p d t (h u) -> p d t h u", h=2)[:, :, :, :, 64:65], 1.0)
            for d in range(NCORE):
                psV = ps.tile([P, CHUNK], f32, tag="bank")
                for g in range(NG):
                    nc.tensor.matmul(psV, wv_sb[:, g, P * d:P * (d + 1)],
                                     hT_sb[:, g, :], start=(g == 0),
                                     stop=(g == NG - 1))
                vt = lnp.tile([P, CHUNK], bf16, tag="vt")
                nc.vector.tensor_scalar_add(vt, psV, bv_sb[:, d:d + 1])
                for tt in range(NTT):
                    tpv = ps.tile([P, P], bf16, tag="bank")
                    nc.tensor.transpose(tpv, vt[:, P * tt:P * (tt + 1)], ident)
                    nc.vector.tensor_copy(
                        vT_st[:, d, tt, :].rearrange("p (h u) -> p h u",
                                                     h=2)[:, :, 0:64],
                        tpv.rearrange("p (h u) -> p h u", h=2))
            nc.sync.dma_start(
                out=v_b[:, :, :, :].rearrange("d p t c -> p d t c"),
                in_=vT_st)
            cc_v = nc.gpsimd.collective_compute(
                "AllToAll", Alu.bypass, replica_groups=groups,
                ins=[v_b[:, :, :, :]], outs=[v_r[:, :, :, :]])

            # phase 1: q/k of even heads (head 2d -> dst core d)
            wq1_sb = wqp.tile([P, NG, C], bf16, tag="wblk", name="wq1_sb")
            nc.sync.dma_start(out=wq1_sb, in_=wqkv[2])
            qk0_st = stg.tile([P, NCORE, CHUNK], bf16, tag="stg",
                              name="qk0_st")
            for d in range(NCORE):
                psA = ps.tile([P, CHUNK], f32, tag="bank")
                for g in range(NG):
                    nc.tensor.matmul(psA, wq0_sb[:, g, P * d:P * (d + 1)],
                                     hT_sb[:, g, :], start=(g == 0),
                                     stop=(g == NG - 1))
                nc.vector.tensor_scalar_add(qk0_st[:, d, :], psA,
                                            bqk0_sb[:, d:d + 1])
            nc.sync.dma_start(
                out=qk0_b[:, :, :].rearrange("d p m -> p d m"),
                in_=qk0_st)
            nc.gpsimd.collective_compute(
                "AllToAll", Alu.bypass, replica_groups=groups,
                ins=[qk0_b[:, :, :]], outs=[qk0_r[:, :, :]])

            # phase 2: q/k of odd heads
            qk1_st = stg.tile([P, NCORE, CHUNK], bf16, tag="stg",
                              name="qk1_st")
            for d in range(NCORE):
                psA = ps.tile([P, CHUNK], f32, tag="bank")
                for g in range(NG):
                    nc.tensor.matmul(psA, wq1_sb[:, g, P * d:P * (d + 1)],
                                     hT_sb[:, g, :], start=(g == 0),
                                     stop=(g == NG - 1))
                nc.vector.tensor_scalar_add(qk1_st[:, d, :], psA,
                                            bqk1_sb[:, d:d + 1])
            nc.sync.dma_start(
                out=qk1_b[:, :, :].rearrange("d p m -> p d m"),
                in_=qk1_st)
            cc_qk1 = nc.gpsimd.collective_compute(
                "AllToAll", Alu.bypass, replica_groups=groups,
                ins=[qk1_b[:, :, :]], outs=[qk1_r[:, :, :]])
            # w1 prefetch: held behind the last QKV AllToAll so this 8MB
            # load runs during attention, clear of startup and collectives
            w1_dma = nc.gpsimd.dma_start(out=w1_sb, in_=w1)
            add_dep_helper(w1_dma.ins, cc_qk1.ins, sync=True,
                           reason="w1 load during attention")

        # ---------------- Stage B: attention --------------------------------
        with ExitStack() as sb:
            qkp = sb.enter_context(tc.tile_pool(name="qkp", bufs=1))

            qT = [qkp.tile([64, TOK], bf16, name=f"qT{i}") for i in range(LH)]
            kT = [qkp.tile([64, TOK], bf16, name=f"kT{i}") for i in range(LH)]
            Vsb = qkp.tile([P, TOK // P, 132], bf16)

            # V + even-head q/k land first; their loads ride the scalar
            # queue (idle between LN1 and the first exp). Odd-head loads
            # ride sync.
            nc.scalar.dma_start(
                out=Vsb.rearrange("p (d t) c -> p d t c", d=NCORE),
                in_=v_r[:, :, :, :].rearrange("d p t c -> p d t c"))
            for hp, qk_r in ((0, qk0_r), (1, qk1_r)):
                eng = nc.scalar if hp == 0 else nc.sync
                eng.dma_start(
                    out=qT[hp].rearrange("p (d m) -> p d m", d=NCORE),
                    in_=qk_r[:, 0:64, :].rearrange("d p m -> p d m"))
                eng.dma_start(
                    out=kT[hp].rearrange("p (d m) -> p d m", d=NCORE),
                    in_=qk_r[:, 64:128, :].rearrange("d p m -> p d m"))
            if DEBUG:
                for hp in range(LH):
                    nc.scalar.dma_start(out=dbg_qkT[hp, 0], in_=qT[hp])
                    nc.scalar.dma_start(out=dbg_qkT[hp, 1], in_=kT[hp])
                nc.scalar.dma_start(out=dbg_v, in_=Vsb)

            # attention: per local head hp, batch b, query tile jq (512 wide)
            atp = sb.enter_context(tc.tile_pool(name="atp", bufs=4))
            ate = sb.enter_context(tc.tile_pool(name="ate", bufs=2))
            for hp in range(LH):
                for b in range(B):
                    base_t = T * b
                    for jq in range(4):
                        q0 = base_t + 512 * jq
                        nk = 4 * (jq + 1)
                        psPV = ps.tile([65, 512], f32, tag="bank")
                        for ik in range(nk):
                            k0 = base_t + P * ik
                            psS = ps2.tile([P, 512], f32, tag="bank2")
                            nc.tensor.matmul(
                                psS, kT[hp][:, k0:k0 + P],
                                qT[hp][:, q0:q0 + 512],
                                start=True, stop=True)
                            pt = atp.tile([P, 512], bf16, tag="pt")
                            nc.scalar.activation(pt, psS, Act.Exp)
                            m = ik - 4 * jq
                            if m >= 0:  # diagonal block: causal mask
                                nc.vector.tensor_mul(pt, pt, masks[:, m, :])
                            nc.tensor.matmul(
                                psPV, Vsb[:, (base_t // P) + ik,
                                          66 * hp:66 * hp + 65],
                                pt, start=(ik == 0), stop=(ik == nk - 1))
                        rs = ate.tile([1, 512], f32, tag="rs")
                        nc.vector.tensor_copy(rs, psPV[64:65, :])
                        rec_f = ate.tile([1, 512], f32, tag="rec_f")
                        nc.vector.reciprocal_approx_fast(rec_f, rs)
                        bc = ate.tile([64, 512], f32, tag="bc")
                        nc.gpsimd.partition_broadcast(bc, rec_f[0:1, :])
                        att = ate.tile([64, 512], bf16, tag="attout")
                        nc.vector.tensor_mul(att, psPV[0:64, :], bc)
                        nc.gpsimd.dma_start(
                            out=attT_bounce[hp][4 * b + jq, :, :],
                            in_=att)
                if b == B - 1:
                    nc.gpsimd.collective_compute(
                        "AllToAll", Alu.bypass, replica_groups=groups,
                        ins=[attT_bounce[hp][:, :, :]],
                        outs=[attT_recv[hp][:, :, :]])
            tc.no_sync_barrier()

        # ---------------- Stage C: proj + residual --------------------------
        # Wproj rows host-permuted: first 512 = even-head channels, last 512
        # = odd. The even half only needs attT_recv[0], so it runs while the
        # second AllToAll flies.
        with ExitStack() as sc:
            wp_dma = nc.sync.dma_start(out=wp, in_=wproj)
            add_dep_helper(wp_dma.ins, cc_qk1.ins, sync=True,
                           reason="wproj load during attention")
            for hp, dstt in ((0, ats0), (1, ats1)):
                rv = attT_recv[hp][:, :, :].rearrange(
                    "(gg two) p m -> p gg two m", two=2)
                nc.sync.dma_start(out=dstt[0:64, :, :], in_=rv[:, :, 0, :])
                nc.sync.dma_start(out=dstt[64:128, :, :], in_=rv[:, :, 1, :])
            # 8 accumulators: 4 single-bank + 2 double-bank halves
            pA = [ps.tile([P, 512], f32, tag="bank", name=f"prA{i}")
                  for i in range(4)]
            pB = [ps2.tile([P, 512], f32, tag="bank2", name=f"prB{i}")
                  for i in range(4)]
            acc = pA + pB
            for half, srct in ((0, ats0), (1, ats1)):
                for n in range(2):
                    for jt in range(NTT):
                        for gg in range(4):
                            nc.tensor.matmul(
                                acc[4 * n + jt],
                                srct[:, gg, P * jt:P * (jt + 1)],
                                wp[:, 4 * half + gg, 512 * n:512 * (n + 1)],
                                start=(half == 0 and gg == 0),
                                stop=(half == 1 and gg == 3))
            for n in range(2):
                for jt in range(NTT):
                    nc.vector.tensor_add(
                        xmid_sb[:, jt, 512 * n:512 * (n + 1)], acc[4 * n + jt],
                        xc_sb[:, jt, 512 * n:512 * (n + 1)])

        if DEBUG:
            nc.sync.dma_start(out=dbg_xmid, in_=xmid_sb)

        # ---------------- Stage D: LN2 + FFN + residual ---------------------
        with ExitStack() as sd:
            ffp = sd.enter_context(tc.tile_pool(name="ffp", bufs=1))
            lnp2 = sd.enter_context(tc.tile_pool(name="lnp2", bufs=3))
            w2p = sd.enter_context(tc.tile_pool(name="w2p", bufs=2))
            outp = sd.enter_context(tc.tile_pool(name="outp", bufs=3))

            h2T = ffp.tile([P, NG, CHUNK], bf16)
            ff1T = ffp.tile([P, NF, CHUNK], bf16)

            for jt in range(NTT):
                hn2 = layernorm_tile(lnp2, xmid_sb[:, jt, :], bf16)
                for g in range(NG):
                    tp = ps.tile([P, P], bf16, tag="bank")
                    nc.tensor.transpose(tp, hn2[:, P * g:P * (g + 1)], ident)
                    nc.vector.tensor_copy(
                        h2T[:, g, P * jt:P * (jt + 1)], tp)

            for f in range(NF):
                psF = ps2.tile([P, CHUNK], f32, tag="bank2")
                for g in range(NG):
                    nc.tensor.matmul(psF, w1_sb[:, g, P * f:P * (f + 1)],
                                     h2T[:, g, :],
                                     start=(g == 0), stop=(g == NG - 1))
                nc.scalar.activation(ff1T[:, f, :], psF, Act.Relu,
                                     bias=bff1_sb[:, f:f + 1])

            # FFN2: w2 loaded in 1MB quarters, double-buffered
            for n in range(2):
                psj = [ps.tile([P, 512], f32, tag="bank", name=f"psk{n}_{jt}")
                       for jt in range(NTT)]
                for a in range(4):
                    w2q = w2p.tile([P, 8, 512], bf16, tag="w2q")
                    nc.sync.dma_start(out=w2q, in_=w2[n, a])
                    if a < 3:
                        for ql in range(8):
                            q = 8 * a + ql
                            for jt in range(NTT):
                                nc.tensor.matmul(
                                    psj[jt], ff1T[:, q, P * jt:P * (jt + 1)],
                                    w2q[:, ql, :], start=(q == 0), stop=False)
                    else:
                        # jt-major so early tiles finish; add+store overlap
                        for jt in range(NTT):
                            for ql in range(8):
                                nc.tensor.matmul(
                                    psj[jt],
                                    ff1T[:, 8 * a + ql,
                                         P * jt:P * (jt + 1)],
                                    w2q[:, ql, :], start=False,
                                    stop=(ql == 7))
                            ot = outp.tile([P, 512], f32, tag="outt")
                            nc.vector.tensor_add(
                                ot, psj[jt],
                                xmid_sb[:, jt, 512 * n:512 * (n + 1)])
                            nc.gpsimd.dma_start(
                                out=out[P * jt:P * (jt + 1),
                                        512 * n:512 * (n + 1)],
                                in_=ot)

    nc.compile()
    return nc


def _pack_pg(w):
    """[C, M] -> [P, C//P, M] partition-major packing."""
    Cr, M = w.shape
    return np.ascontiguousarray(
        w.reshape(Cr // P, P, M).transpose(1, 0, 2))


def _prepare_inputs(x, Wq, Wk, Wv, p, Wproj, W1, W2,
                    ln1_w, ln1_b, ln2_w, ln2_b):
    import ml_dtypes
    f = np.float32
    bf = ml_dtypes.bfloat16
    x = np.asarray(x, f).reshape(TOK, C)
    Wq, Wk, Wv = (np.asarray(a, f) for a in (Wq, Wk, Wv))
    p = np.asarray(p, f)
    Wproj = np.asarray(Wproj, f)
    W1, W2 = np.asarray(W1, f), np.asarray(W2, f)
    ln1_w, ln1_b = np.asarray(ln1_w, f), np.asarray(ln1_b, f)
    ln2_w, ln2_b = np.asarray(ln2_w, f), np.asarray(ln2_b, f)

    s = (p.astype(np.float64) ** -0.5).astype(f)

    w1_p = _pack_pg((ln2_w[:, None] * W1).astype(bf))
    bff1 = ln2_b @ W1
    bff1 = np.ascontiguousarray(bff1.reshape(NF, P).T.astype(f))
    # w2 quarters: [2 n, 4 a, P, 8, 512]
    w2_bf = W2.astype(bf)
    w2_p = np.empty((2, 4, P, 8, 512), bf)
    for n in range(2):
        for a in range(4):
            blk = w2_bf[1024 * a:1024 * (a + 1), 512 * n:512 * (n + 1)]
            w2_p[n, a] = blk.reshape(8, P, 512).transpose(1, 0, 2)
    # Wproj rows permuted: even-head channels first, then odd
    ev = np.arange(C).reshape(H, HS)[0::2].ravel()
    od = np.arange(C).reshape(H, HS)[1::2].ravel()
    wproj_p = _pack_pg(
        np.concatenate([Wproj[ev], Wproj[od]], axis=0).astype(bf))

    # wqkv blocks: [v | qk-even | qk-odd], packed per block
    vblk = np.concatenate(
        [np.concatenate([ln1_w[:, None] * Wv[2 * d],
                         ln1_w[:, None] * Wv[2 * d + 1]], axis=1)
         for d in range(NCORE)], axis=1)
    qk0blk = np.concatenate(
        [np.concatenate([ln1_w[:, None] * Wq[2 * d] * s[2 * d],
                         ln1_w[:, None] * Wk[2 * d]], axis=1)
         for d in range(NCORE)], axis=1)
    qk1blk = np.concatenate(
        [np.concatenate([ln1_w[:, None] * Wq[2 * d + 1] * s[2 * d + 1],
                         ln1_w[:, None] * Wk[2 * d + 1]], axis=1)
         for d in range(NCORE)], axis=1)
    wqkv_p = np.stack([_pack_pg(vblk.astype(bf)),
                       _pack_pg(qk0blk.astype(bf)),
                       _pack_pg(qk1blk.astype(bf))])

    # K bias intentionally zero: softmax is invariant to it
    bqk0 = np.stack([np.concatenate([s[2 * d] * (ln1_b @ Wq[2 * d]),
                                     np.zeros(HS, f)]) for d in range(NCORE)],
                    axis=1)
    bqk1 = np.stack([np.concatenate([s[2 * d + 1] * (ln1_b @ Wq[2 * d + 1]),
                                     np.zeros(HS, f)]) for d in range(NCORE)],
                    axis=1)
    bv_a = np.stack([np.concatenate([ln1_b @ Wv[2 * d],
                                     ln1_b @ Wv[2 * d + 1]])
                     for d in range(NCORE)], axis=1)

    shared = {
        "wqkv": np.ascontiguousarray(wqkv_p),
        "bqk0": np.ascontiguousarray(bqk0.astype(f)),
        "bqk1": np.ascontiguousarray(bqk1.astype(f)),
        "bv": np.ascontiguousarray(bv_a.astype(f)),
        "wproj": wproj_p,
        "w1": w1_p,
        "bff1": bff1,
        "w2": np.ascontiguousarray(w2_p),
    }
    in_maps = []
    for c in range(NCORE):
        m = dict(shared)
        xch = x[CHUNK * c:CHUNK * (c + 1)]
        m["xc"] = np.ascontiguousarray(
            xch.reshape(NTT, P, C).transpose(1, 0, 2))
        in_maps.append(m)
    return in_maps


def kernel(**inputs):
    global _BUILT
    from concourse.bass_utils import run_bass_kernel_spmd

    if _BUILT is None:
        _BUILT = _build()
    in_maps = _prepare_inputs(**inputs)
    trace = bool(int(os.environ.get("BASSK_TRACE", "0")))
    res = run_bass_kernel_spmd(_BUILT, in_maps, list(range(NCORE)),
                               trace=trace)
    if trace:
        kernel.last_exec_time_ns = res.exec_time_ns
        kernel.last_res = res
    out = np.concatenate([res.results[c]["out"] for c in range(NCORE)], axis=0)
    return out.reshape(B, T, C).astype(np.float32)


# revision 22
# speedup vs baseline: 1.0749x; 1.0749x over previous
"""Trainium2 Bass kernel for a dense transformer block (B=2, T=2048, C=1024,
H=16 heads, HS=64, FF=4096, fp32), SPMD across 8 NeuronCores.

Sharding strategy (v2)
----------------------
- LayerNorms + FFN + proj: sequence-parallel; core c owns 512 tokens.
- Attention: head-parallel; core c owns heads 2c, 2c+1 over all tokens.
- QKV is computed token-sharded (each core projects its OWN 512 tokens
  through ALL 16 heads' Q/K/V), so the PE has work during the runtime's
  collective-stream init barrier; results are re-sharded head-wise with
  three pipelined 1MB AllToAlls (V^T first - PV needs it earliest - then
  qk-even-heads, qk-odd-heads) instead of one serial 8MB AllGather.
- att^T returns to token-sharding with one AllToAll per local head; the
  even-head half of the output projection (host-permuted Wproj rows)
  overlaps the second AllToAll.

Numerics: matmul operands bf16 (fp32 PSUM accumulate); LayerNorm stats,
softmax normalization, residuals in fp32. LN scale/bias and the per-head
attention scale fold into the weights on the host; K-bias dropped
(softmax is invariant to per-query constant offsets).

All weights and x arrive host-packed in [partition, tile, cols] layout so
every DMA is 128 large contiguous descriptors (no rearrange storms).
Causal masking is a DVE multiply with four precomputed [128,512] 0/1
tiles; the softmax denominator reciprocal is broadcast across partitions
with a GpSimd partition_broadcast.
"""

import os
import numpy as np

B, T, C = 2, 2048, 1024
H, HS = 16, 64
FF = 4 * C
EPS = 1e-5
NCORE = 8
TOK = B * T            # 4096 flattened tokens
CHUNK = TOK // NCORE   # 512 tokens per core
P = 128
NTT = CHUNK // P       # 4 token tiles of 128 per core
NG = C // P            # 8 channel chunks
NF = FF // P           # 32 ff slices
LH = 2                 # local heads per core

_BUILT = None


def _build():
    import concourse.bass as bass
    import concourse.tile as tile
    from bass_rust import add_dep_helper
    from concourse import bacc, mybir
    from concourse.masks import make_identity
    from contextlib import ExitStack

    f32 = mybir.dt.float32
    bf16 = mybir.dt.bfloat16
    Alu = mybir.AluOpType
    Act = mybir.ActivationFunctionType

    nc = bacc.Bacc("TRN2", target_bir_lowering=False, debug=False,
                   num_devices=NCORE)

    xc = nc.dram_tensor("xc", [P, NTT, C], f32, kind="ExternalInput").ap()
    # wqkv blocks: [0]=v (heads 2d,2d+1 per dst d), [1]=qk head 2d, [2]=qk 2d+1
    wqkv = nc.dram_tensor("wqkv", [3, P, NG, C], bf16,
                          kind="ExternalInput").ap()
    bqk0 = nc.dram_tensor("bqk0", [P, NCORE], f32, kind="ExternalInput").ap()
    bqk1 = nc.dram_tensor("bqk1", [P, NCORE], f32, kind="ExternalInput").ap()
    bv = nc.dram_tensor("bv", [P, NCORE], f32, kind="ExternalInput").ap()
    wproj = nc.dram_tensor("wproj", [P, NG, C], bf16,
                           kind="ExternalInput").ap()
    w1 = nc.dram_tensor("w1", [P, NG, FF], bf16, kind="ExternalInput").ap()
    bff1 = nc.dram_tensor("bff1", [P, NF], f32, kind="ExternalInput").ap()
    w2 = nc.dram_tensor("w2", [2, 4, P, 8, 512], bf16,
                        kind="ExternalInput").ap()
    out = nc.dram_tensor("out", [CHUNK, C], f32, kind="ExternalOutput").ap()
    DEBUG = bool(int(os.environ.get("BASSK_DEBUG", "0")))
    if DEBUG:
        dbg_qkT = nc.dram_tensor("dbg_qkT", [2, 2, 64, TOK], bf16,
                                 kind="ExternalOutput").ap()
        dbg_v = nc.dram_tensor("dbg_v", [P, TOK // P, 132], bf16,
                               kind="ExternalOutput").ap()
        dbg_xmid = nc.dram_tensor("dbg_xmid", [P, NTT, C], f32,
                                  kind="ExternalOutput").ap()

    qk0_b = nc.dram_tensor("qk0_b", [NCORE, P, CHUNK], bf16)
    qk1_b = nc.dram_tensor("qk1_b", [NCORE, P, CHUNK], bf16)
    v_b = nc.dram_tensor("v_b", [NCORE, P, NTT, 132], bf16)
    qk0_r = nc.dram_tensor("qk0_r", [NCORE, P, CHUNK], bf16)
    qk1_r = nc.dram_tensor("qk1_r", [NCORE, P, CHUNK], bf16)
    v_r = nc.dram_tensor("v_r", [NCORE, P, NTT, 132], bf16)
    attT_bounce = [nc.dram_tensor(f"attT_bounce{i}", [NCORE, 64, CHUNK], bf16)
                   for i in range(LH)]
    attT_recv = [nc.dram_tensor(f"attT_recv{i}", [NCORE, 64, CHUNK], bf16)
                 for i in range(LH)]
    groups = [list(range(NCORE))]

    with tile.TileContext(nc) as tc, ExitStack() as top:
        const = top.enter_context(tc.tile_pool(name="const", bufs=1))
        persist = top.enter_context(tc.tile_pool(name="persist", bufs=1))
        ps = top.enter_context(tc.tile_pool(name="ps", bufs=4, space="PSUM"))
        ps2 = top.enter_context(tc.tile_pool(name="ps2", bufs=2, space="PSUM"))

        ident = const.tile([P, P], bf16)
        make_identity(nc, ident)
        eps_sb = const.tile([P, 1], f32)
        nc.vector.memset(eps_sb, EPS)
        # causal mask tiles: mask[m][p, col] = col >= p + 128*m
        masks = const.tile([P, 4, 512], bf16)
        nc.vector.memset(masks, 1.0)
        for m in range(4):
            nc.gpsimd.affine_select(
                out=masks[:, m, :], in_=masks[:, m, :], pattern=[[1, 512]],
                compare_op=Alu.is_ge, fill=0.0, base=-128 * m,
                channel_multiplier=-1)

        xc_sb = persist.tile([P, NTT, C], f32)
        xmid_sb = persist.tile([P, NTT, C], f32)
        bqk0_sb = persist.tile([P, NCORE], f32)
        bqk1_sb = persist.tile([P, NCORE], f32)
        bv_sb = persist.tile([P, NCORE], f32)
        bff1_sb = persist.tile([P, NF], f32)
        w1p = top.enter_context(tc.tile_pool(name="w1p", bufs=1))
        w1_sb = w1p.tile([P, NG, FF], bf16)   # prefetched during attention
        prp = top.enter_context(tc.tile_pool(name="prp", bufs=1))
        wpp = top.enter_context(tc.tile_pool(name="wpp", bufs=1))
        ats0 = prp.tile([P, 4, CHUNK], bf16, name="ats0")
        ats1 = prp.tile([P, 4, CHUNK], bf16, name="ats1")
        wp = wpp.tile([P, NG, C], bf16)

        for jt in range(NTT):
            nc.sync.dma_start(out=xc_sb[:, jt, :], in_=xc[:, jt, :])
        nc.sync.dma_start(out=bqk0_sb, in_=bqk0)
        nc.sync.dma_start(out=bqk1_sb, in_=bqk1)
        nc.sync.dma_start(out=bv_sb, in_=bv)
        nc.sync.dma_start(out=bff1_sb, in_=bff1)

        def layernorm_tile(pool, src_ap, out_dt):
            """src_ap: [P, C] fp32 in SBUF -> normalized [P, C] tile."""
            stats = pool.tile([P, 2, 6], f32, tag="ln_stats")
            nc.vector.bn_stats(out=stats[:, 0, :], in_=src_ap[:, 0:512])
            nc.vector.bn_stats(out=stats[:, 1, :], in_=src_ap[:, 512:1024])
            mv = pool.tile([P, 2], f32, tag="ln_mv")
            nc.vector.bn_aggr(out=mv, in_=stats)
            rstd = pool.tile([P, 1], f32, tag="ln_rstd")
            nc.scalar.activation(rstd, mv[:, 1:2], Act.Sqrt, bias=eps_sb)
            nc.vector.reciprocal(rstd, rstd)
            negmr = pool.tile([P, 1], f32, tag="ln_negmr")
            nc.vector.tensor_scalar(negmr, mv[:, 0:1], rstd, -1.0,
                                    Alu.mult, Alu.mult)
            hn = pool.tile([P, C], out_dt, tag="ln_out")
            nc.scalar.activation(hn, src_ap, Act.Identity,
                                 bias=negmr, scale=rstd)
            return hn

        # ---------------- Stage A: LN1 + transpose (own chunk) --------------
        with ExitStack() as sa:
            lnp = sa.enter_context(tc.tile_pool(name="lnp", bufs=3))
            qkvp = sa.enter_context(tc.tile_pool(name="qkvp", bufs=1))
            wqp = sa.enter_context(tc.tile_pool(name="wqp", bufs=2))

            hT_sb = qkvp.tile([P, NG, CHUNK], bf16)
            wv_sb = wqp.tile([P, NG, C], bf16, tag="wblk", name="wv_sb")
            nc.sync.dma_start(out=wv_sb, in_=wqkv[0])
            wq0_sb = wqp.tile([P, NG, C], bf16, tag="wblk", name="wq0_sb")
            nc.sync.dma_start(out=wq0_sb, in_=wqkv[1])

            for jt in range(NTT):
                hn = layernorm_tile(lnp, xc_sb[:, jt, :], bf16)
                for g in range(NG):
                    tp = ps.tile([P, P], bf16, tag="bank")
                    nc.tensor.transpose(tp, hn[:, P * g:P * (g + 1)], ident)
                    nc.vector.tensor_copy(
                        hT_sb[:, g, P * jt:P * (jt + 1)], tp)

            # -------- Stage A2: QKV for all heads, 3 pipelined AllToAlls ----
            stg = sa.enter_context(tc.tile_pool(name="stg", bufs=2))

            # phase 0: V (both heads of dst d), pre-transposed to [token, ch]
            vT_st = stg.tile([P, NCORE, NTT, 132], bf16, tag="stg",
                             name="vT_st")
            nc.vector.memset(vT_st.rearrange(
                "p d t (h u) -> p d t h u", h=2)[:, :, :, :, 64:65], 1.0)
            for d in range(NCORE):
                psV = ps.tile([P, CHUNK], f32, tag="bank")
                for g in range(NG):
                    nc.tensor.matmul(psV, wv_sb[:, g, P * d:P * (d + 1)],
                                     hT_sb[:, g, :], start=(g == 0),
                                     stop=(g == NG - 1))
                vt = lnp.tile([P, CHUNK], bf16, tag="vt")
                nc.vector.tensor_scalar_add(vt, psV, bv_sb[:, d:d + 1])
                for tt in range(NTT):
                    tpv = ps.tile([P, P], bf16, tag="bank")
                    nc.tensor.transpose(tpv, vt[:, P * tt:P * (tt + 1)], ident)
                    nc.vector.tensor_copy(
                        vT_st[:, d, tt, :].rearrange("p (h u) -> p h u",
                                                     h=2)[:, :, 0:64],
                        tpv.rearrange("p (h u) -> p h u", h=2))
            nc.sync.dma_start(
                out=v_b[:, :, :, :].rearrange("d p t c -> p d t c"),
                in_=vT_st)
            cc_v = nc.gpsimd.collective_compute(
                "AllToAll", Alu.bypass, replica_groups=groups,
                ins=[v_b[:, :, :, :]], outs=[v_r[:, :, :, :]])

            # phase 1: q/k of even heads (head 2d -> dst core d)
            wq1_sb = wqp.tile([P, NG, C], bf16, tag="wblk", name="wq1_sb")
            nc.sync.dma_start(out=wq1_sb, in_=wqkv[2])
            qk0_st = stg.tile([P, NCORE, CHUNK], bf16, tag="stg",
                              name="qk0_st")
            for d in range(NCORE):
                psA = ps.tile([P, CHUNK], f32, tag="bank")
                for g in range(NG):
                    nc.tensor.matmul(psA, wq0_sb[:, g, P * d:P * (d + 1)],
                                     hT_sb[:, g, :], start=(g == 0),
                                     stop=(g == NG - 1))
                nc.vector.tensor_scalar_add(qk0_st[:, d, :], psA,
                                            bqk0_sb[:, d:d + 1])
            nc.sync.dma_start(
                out=qk0_b[:, :, :].rearrange("d p m -> p d m"),
                in_=qk0_st)
            nc.gpsimd.collective_compute(
                "AllToAll", Alu.bypass, replica_groups=groups,
                ins=[qk0_b[:, :, :]], outs=[qk0_r[:, :, :]])

            # phase 2: q/k of odd heads
            qk1_st = stg.tile([P, NCORE, CHUNK], bf16, tag="stg",
                              name="qk1_st")
            for d in range(NCORE):
                psA = ps.tile([P, CHUNK], f32, tag="bank")
                for g in range(NG):
                    nc.tensor.matmul(psA, wq1_sb[:, g, P * d:P * (d + 1)],
                                     hT_sb[:, g, :], start=(g == 0),
                                     stop=(g == NG - 1))
                nc.vector.tensor_scalar_add(qk1_st[:, d, :], psA,
                                            bqk1_sb[:, d:d + 1])
            nc.sync.dma_start(
                out=qk1_b[:, :, :].rearrange("d p m -> p d m"),
                in_=qk1_st)
            cc_qk1 = nc.gpsimd.collective_compute(
                "AllToAll", Alu.bypass, replica_groups=groups,
                ins=[qk1_b[:, :, :]], outs=[qk1_r[:, :, :]])
            # w1 prefetch: held behind the last QKV AllToAll so this 8MB
            # load runs during attention, clear of startup and collectives
            w1_dma = nc.gpsimd.dma_start(out=w1_sb, in_=w1)
            add_dep_helper(w1_dma.ins, cc_qk1.ins, sync=True,
                           reason="w1 load during attention")

        # ---------------- Stage B: attention --------------------------------
        with ExitStack() as sb:
            qkp = sb.enter_context(tc.tile_pool(name="qkp", bufs=1))

            qT = [qkp.tile([64, TOK], bf16, name=f"qT{i}") for i in range(LH)]
            kT = [qkp.tile([64, TOK], bf16, name=f"kT{i}") for i in range(LH)]
            Vsb = qkp.tile([P, TOK // P, 132], bf16)

            # V + even-head q/k land first; their loads ride the scalar
            # queue (idle between LN1 and the first exp). Odd-head loads
            # ride sync.
            nc.scalar.dma_start(
                out=Vsb.rearrange("p (d t) c -> p d t c", d=NCORE),
                in_=v_r[:, :, :, :].rearrange("d p t c -> p d t c"))
            for hp, qk_r in ((0, qk0_r), (1, qk1_r)):
                eng = nc.scalar if hp == 0 else nc.sync
                eng.dma_start(
                    out=qT[hp].rearrange("p (d m) -> p d m", d=NCORE),
                    in_=qk_r[:, 0:64, :].rearrange("d p m -> p d m"))
                eng.dma_start(
                    out=kT[hp].rearrange("p (d m) -> p d m", d=NCORE),
                    in_=qk_r[:, 64:128, :].rearrange("d p m -> p d m"))
            if DEBUG:
                for hp in range(LH):
                    nc.scalar.dma_start(out=dbg_qkT[hp, 0], in_=qT[hp])
                    nc.scalar.dma_start(out=dbg_qkT[hp, 1], in_=kT[hp])
                nc.scalar.dma_start(out=dbg_v, in_=Vsb)

            # attention: per local head hp, batch b, query tile jq (512 wide)
            atp = sb.enter_context(tc.tile_pool(name="atp", bufs=4))
            ate = sb.enter_context(tc.tile_pool(name="ate", bufs=2))
            for hp in range(LH):
                for b in range(B):
                    base_t = T * b
                    for jq in range(4):
                        q0 = base_t + 512 * jq
                        nk = 4 * (jq + 1)
                        psPV = ps.tile([65, 512], f32, tag="bank")
                        for ike in range(0, nk, 2):
                            psS2 = ps2.tile([P, 1024], f32, tag="bank2")
                            for dd in range(2):
                                ik = ike + dd
                                k0 = base_t + P * ik
                                nc.tensor.matmul(
                                    psS2[:, 512 * dd:512 * (dd + 1)],
                                    kT[hp][:, k0:k0 + P],
                                    qT[hp][:, q0:q0 + 512],
                                    start=True, stop=True)
                            pt = atp.tile([P, 1024], bf16, tag="pt")
                            nc.scalar.activation(pt, psS2, Act.Exp)
                            for dd in range(2):
                                ik = ike + dd
                                ph = pt[:, 512 * dd:512 * (dd + 1)]
                                m = ik - 4 * jq
                                if m >= 0:  # diagonal block: causal mask
                                    nc.vector.tensor_mul(ph, ph,
                                                         masks[:, m, :])
                                nc.tensor.matmul(
                                    psPV, Vsb[:, (base_t // P) + ik,
                                              66 * hp:66 * hp + 65],
                                    ph, start=(ik == 0), stop=(ik == nk - 1))
                        rs = ate.tile([1, 512], f32, tag="rs")
                        nc.vector.tensor_copy(rs, psPV[64:65, :])
                        rec_f = ate.tile([1, 512], f32, tag="rec_f")
                        nc.vector.reciprocal_approx_fast(rec_f, rs)
                        bc = ate.tile([64, 512], f32, tag="bc")
                        nc.gpsimd.partition_broadcast(bc, rec_f[0:1, :])
                        att = ate.tile([64, 512], bf16, tag="attout")
                        nc.vector.tensor_mul(att, psPV[0:64, :], bc)
                        nc.gpsimd.dma_start(
                            out=attT_bounce[hp][4 * b + jq, :, :],
                            in_=att)
                if b == B - 1:
                    nc.gpsimd.collective_compute(
                        "AllToAll", Alu.bypass, replica_groups=groups,
                        ins=[attT_bounce[hp][:, :, :]],
                        outs=[attT_recv[hp][:, :, :]])
            tc.no_sync_barrier()

        # ---------------- Stage C: proj + residual --------------------------
        # Wproj rows host-permuted: first 512 = even-head channels, last 512
        # = odd. The even half only needs attT_recv[0], so it runs while the
        # second AllToAll flies.
        with ExitStack() as sc:
            wp_dma = nc.sync.dma_start(out=wp, in_=wproj)
            add_dep_helper(wp_dma.ins, cc_qk1.ins, sync=True,
                           reason="wproj load during attention")
            for hp, dstt in ((0, ats0), (1, ats1)):
                rv = attT_recv[hp][:, :, :].rearrange(
                    "(gg two) p m -> p gg two m", two=2)
                nc.sync.dma_start(out=dstt[0:64, :, :], in_=rv[:, :, 0, :])
                nc.sync.dma_start(out=dstt[64:128, :, :], in_=rv[:, :, 1, :])
            # 8 accumulators: 4 single-bank + 2 double-bank halves
            pA = [ps.tile([P, 512], f32, tag="bank", name=f"prA{i}")
                  for i in range(4)]
            pB = [ps2.tile([P, 1024], f32, tag="bank2", name=f"prB{i}")
                  for i in range(2)]
            acc = pA + [pB[0][:, 0:512], pB[0][:, 512:1024],
                        pB[1][:, 0:512], pB[1][:, 512:1024]]
            for half, srct in ((0, ats0), (1, ats1)):
                for n in range(2):
                    for jt in range(NTT):
                        for gg in range(4):
                            nc.tensor.matmul(
                                acc[4 * n + jt],
                                srct[:, gg, P * jt:P * (jt + 1)],
                                wp[:, 4 * half + gg, 512 * n:512 * (n + 1)],
                                start=(half == 0 and gg == 0),
                                stop=(half == 1 and gg == 3))
            for n in range(2):
                for jt in range(NTT):
                    nc.vector.tensor_add(
                        xmid_sb[:, jt, 512 * n:512 * (n + 1)], acc[4 * n + jt],
                        xc_sb[:, jt, 512 * n:512 * (n + 1)])

        if DEBUG:
            nc.sync.dma_start(out=dbg_xmid, in_=xmid_sb)

        # ---------------- Stage D: LN2 + FFN + residual ---------------------
        with ExitStack() as sd:
            ffp = sd.enter_context(tc.tile_pool(name="ffp", bufs=1))
            lnp2 = sd.enter_context(tc.tile_pool(name="lnp2", bufs=3))
            w2p = sd.enter_context(tc.tile_pool(name="w2p", bufs=2))
            outp = sd.enter_context(tc.tile_pool(name="outp", bufs=3))

            h2T = ffp.tile([P, NG, CHUNK], bf16)
            ff1T = ffp.tile([P, NF, CHUNK], bf16)

            for jt in range(NTT):
                hn2 = layernorm_tile(lnp2, xmid_sb[:, jt, :], bf16)
                for g in range(NG):
                    tp = ps.tile([P, P], bf16, tag="bank")
                    nc.tensor.transpose(tp, hn2[:, P * g:P * (g + 1)], ident)
                    nc.vector.tensor_copy(
                        h2T[:, g, P * jt:P * (jt + 1)], tp)

            for f in range(NF):
                psF = ps2.tile([P, CHUNK], f32, tag="bank2")
                for g in range(NG):
                    nc.tensor.matmul(psF, w1_sb[:, g, P * f:P * (f + 1)],
                                     h2T[:, g, :],
                                     start=(g == 0), stop=(g == NG - 1))
                nc.scalar.activation(ff1T[:, f, :], psF, Act.Relu,
                                     bias=bff1_sb[:, f:f + 1])

            # FFN2: w2 loaded in 1MB quarters, double-buffered
            for n in range(2):
                psj = [ps.tile([P, 512], f32, tag="bank", name=f"psk{n}_{jt}")
                       for jt in range(NTT)]
                for a in range(4):
                    w2q = w2p.tile([P, 8, 512], bf16, tag="w2q")
                    nc.sync.dma_start(out=w2q, in_=w2[n, a])
                    if a < 3:
                        for ql in range(8):
                            q = 8 * a + ql
                            for jt in range(NTT):
                                nc.tensor.matmul(
                                    psj[jt], ff1T[:, q, P * jt:P * (jt + 1)],
                                    w2q[:, ql, :], start=(q == 0), stop=False)
                    else:
                        # jt-major so early tiles finish; add+store overlap
                        for jt in range(NTT):
                            for ql in range(8):
                                nc.tensor.matmul(
                                    psj[jt],
                                    ff1T[:, 8 * a + ql,
                                         P * jt:P * (jt + 1)],
                                    w2q[:, ql, :], start=False,
                                    stop=(ql == 7))
                            ot = outp.tile([P, 512], f32, tag="outt")
                            nc.vector.tensor_add(
                                ot, psj[jt],
                                xmid_sb[:, jt, 512 * n:512 * (n + 1)])
                            nc.gpsimd.dma_start(
                                out=out[P * jt:P * (jt + 1),
                                        512 * n:512 * (n + 1)],
                                in_=ot)

    nc.compile()
    return nc


def _pack_pg(w):
    """[C, M] -> [P, C//P, M] partition-major packing."""
    Cr, M = w.shape
    return np.ascontiguousarray(
        w.reshape(Cr // P, P, M).transpose(1, 0, 2))


def _prepare_inputs(x, Wq, Wk, Wv, p, Wproj, W1, W2,
                    ln1_w, ln1_b, ln2_w, ln2_b):
    import ml_dtypes
    f = np.float32
    bf = ml_dtypes.bfloat16
    x = np.asarray(x, f).reshape(TOK, C)
    Wq, Wk, Wv = (np.asarray(a, f) for a in (Wq, Wk, Wv))
    p = np.asarray(p, f)
    Wproj = np.asarray(Wproj, f)
    W1, W2 = np.asarray(W1, f), np.asarray(W2, f)
    ln1_w, ln1_b = np.asarray(ln1_w, f), np.asarray(ln1_b, f)
    ln2_w, ln2_b = np.asarray(ln2_w, f), np.asarray(ln2_b, f)

    s = (p.astype(np.float64) ** -0.5).astype(f)

    w1_p = _pack_pg((ln2_w[:, None] * W1).astype(bf))
    bff1 = ln2_b @ W1
    bff1 = np.ascontiguousarray(bff1.reshape(NF, P).T.astype(f))
    # w2 quarters: [2 n, 4 a, P, 8, 512]
    w2_bf = W2.astype(bf)
    w2_p = np.empty((2, 4, P, 8, 512), bf)
    for n in range(2):
        for a in range(4):
            blk = w2_bf[1024 * a:1024 * (a + 1), 512 * n:512 * (n + 1)]
            w2_p[n, a] = blk.reshape(8, P, 512).transpose(1, 0, 2)
    # Wproj rows permuted: even-head channels first, then odd
    ev = np.arange(C).reshape(H, HS)[0::2].ravel()
    od = np.arange(C).reshape(H, HS)[1::2].ravel()
    wproj_p = _pack_pg(
        np.concatenate([Wproj[ev], Wproj[od]], axis=0).astype(bf))

    # wqkv blocks: [v | qk-even | qk-odd], packed per block
    vblk = np.concatenate(
        [np.concatenate([ln1_w[:, None] * Wv[2 * d],
                         ln1_w[:, None] * Wv[2 * d + 1]], axis=1)
         for d in range(NCORE)], axis=1)
    qk0blk = np.concatenate(
        [np.concatenate([ln1_w[:, None] * Wq[2 * d] * s[2 * d],
                         ln1_w[:, None] * Wk[2 * d]], axis=1)
         for d in range(NCORE)], axis=1)
    qk1blk = np.concatenate(
        [np.concatenate([ln1_w[:, None] * Wq[2 * d + 1] * s[2 * d + 1],
                         ln1_w[:, None] * Wk[2 * d + 1]], axis=1)
         for d in range(NCORE)], axis=1)
    wqkv_p = np.stack([_pack_pg(vblk.astype(bf)),
                       _pack_pg(qk0blk.astype(bf)),
                       _pack_pg(qk1blk.astype(bf))])

    # K bias intentionally zero: softmax is invariant to it
    bqk0 = np.stack([np.concatenate([s[2 * d] * (ln1_b @ Wq[2 * d]),
                                     np.zeros(HS, f)]) for d in range(NCORE)],
                    axis=1)
    bqk1 = np.stack([np.concatenate([s[2 * d + 1] * (ln1_b @ Wq[2 * d + 1]),
                                     np.zeros(HS, f)]) for d in range(NCORE)],
                    axis=1)
    bv_a = np.stack([np.concatenate([ln1_b @ Wv[2 * d],
                                     ln1_b @ Wv[2 * d + 1]])
                     for d in range(NCORE)], axis=1)

    shared = {
        "wqkv": np.ascontiguousarray(wqkv_p),
        "bqk0": np.ascontiguousarray(bqk0.astype(f)),
        "bqk1": np.ascontiguousarray(bqk1.astype(f)),
        "bv": np.ascontiguousarray(bv_a.astype(f)),
        "wproj": wproj_p,
        "w1": w1_p,
        "bff1": bff1,
        "w2": np.ascontiguousarray(w2_p),
    }
    in_maps = []
    for c in range(NCORE):
        m = dict(shared)
        xch = x[CHUNK * c:CHUNK * (c + 1)]
        m["xc"] = np.ascontiguousarray(
            xch.reshape(NTT, P, C).transpose(1, 0, 2))
        in_maps.append(m)
    return in_maps


def kernel(**inputs):
    global _BUILT
    from concourse.bass_utils import run_bass_kernel_spmd

    if _BUILT is None:
        _BUILT = _build()
    in_maps = _prepare_inputs(**inputs)
    trace = bool(int(os.environ.get("BASSK_TRACE", "0")))
    res = run_bass_kernel_spmd(_BUILT, in_maps, list(range(NCORE)),
                               trace=trace)
    if trace:
        kernel.last_exec_time_ns = res.exec_time_ns
        kernel.last_res = res
    out = np.concatenate([res.results[c]["out"] for c in range(NCORE)], axis=0)
    return out.reshape(B, T, C).astype(np.float32)
